# revision 13
# baseline (speedup 1.0000x reference)
"""Trainium2 Bass kernel for nn_DCTFeatureModel.

Math: the reference pipeline (3D DCT-II over [time-in-bin, H, W], mean over
DCT bins, full-receptive-field Conv3d, bias, LeakyReLU) is linear up to the
LeakyReLU, so everything folds into a single small matmul:

    feat[b,s,o] = LeakyReLU( sum_{c,t,i,j} x[b,s,c,t,i,j] * Weff[s,o,t,i,j]
                             + bias[s,o] )
    Weff[s,o,t,i,j] = (1/8) * sum_{f,p,q} Ct[f,t] Cs[p,i] Cs[q,j] W[s,o,f,p,q]

Weff is tiny and computed on host. The device kernel is memory-bound:
stream x (fp16, 8.4 MB per core), reduce over the 8 DCT bins (c), then a
[128b x 2048k] @ [2048k x 64o] matmul per subwindow.

v5: x ships as 4 x 2 MB group blocks [128, 8*GW], all issued on the sync
queue — measured: this streams back-to-back at ~394 GB/s across the 16 HW
queues with no ring-depth stalls (16 x 512 KB stalled the sequencers; a
gpsimd-issued load never left the ring). Compute: c-pair sums on DVE only
(0.65 us each at the 16-bit rate; GPSIMD adds measured 4x slower), the
remaining 4-way c-reduction rides PE PSUM accumulation (53 ns fp16
matmuls, LDWEIGHTS overlapped), LeakyReLU on DVE, and each subwindow's
output half is DMA'd from the (otherwise idle) gpsimd queue.

Sharding: pure data-parallel over batch, 1024/8 = 128 rows per core.
"""

from contextlib import ExitStack

import numpy as np

import concourse.bacc as bacc
import concourse.tile as tile
from concourse import mybir
from concourse.bass_utils import run_bass_kernel_spmd

# Problem shapes (hardcoded per contract)
B = 1024
NCORES = 8
BS = B // NCORES          # 128 batch rows per core
NSW = 2                   # subwindows
NBINS = 8                 # DCT bins (mean-reduced)
NPAIR = NBINS // 2        # 4 c-pairs per group
NDCT = 32                 # time points per bin
HW = 8
NF = 64                   # conv output filters per subwindow
K = NDCT * HW * HW        # 2048 contraction elements per (s, c)
P = 128                   # partitions
NCHUNK = K // P           # 16 k-chunks of 128
NG = 2                    # chunk-groups per s
CPG = NCHUNK // NG        # 8 chunks per group
GW = CPG * P              # 1024 columns per (chin, b) block
OUT_F = NSW * NF          # 128 output features
SLOPE = 0.02

F32 = mybir.dt.float32
F16 = mybir.dt.float16

_cached = None
last_results = None


def _dct2(N):
    n = np.arange(N, dtype=np.float64)
    k = np.arange(N, dtype=np.float64)
    return 2.0 * np.cos(np.pi * (2.0 * n[None, :] + 1.0) * k[:, None] / (2.0 * N))


def _kernel_body(tc, x, w, bias, out):
    """x: [NSW*NG, 128, NBINS*GW] fp16 — group blocks, cols (c, chin, b)
    w: [P, NSW*NCHUNK*NF] fp16; bias: [1, OUT_F] fp16; out: [BS, OUT_F] fp32"""
    nc = tc.nc
    with ExitStack() as ctx:
        const_pool = ctx.enter_context(tc.tile_pool(name="const", bufs=1))
        xpool = ctx.enter_context(tc.tile_pool(name="xp", bufs=4))
        upool = ctx.enter_context(tc.tile_pool(name="up", bufs=4))
        opool = ctx.enter_context(tc.tile_pool(name="op", bufs=1))
        pft_pool = ctx.enter_context(tc.tile_pool(name="pft", bufs=1, space="PSUM"))

        # consts on the scalar queue so the x stream owns sync
        w_sb = const_pool.tile([P, NSW * NCHUNK * NF], F16)
        nc.scalar.dma_start(out=w_sb, in_=w)
        bias_sb = const_pool.tile([1, OUT_F], F16)
        nc.scalar.dma_start(out=bias_sb, in_=bias)
        ones = const_pool.tile([1, P], F16)
        nc.gpsimd.memset(ones, 1.0)

        out_sb = opool.tile([BS, OUT_F], F32)
        psum_feat = [
            pft_pool.tile([P, NF], F32, tag=f"feat{s}", name=f"psum_feat{s}")
            for s in range(NSW)
        ]

        # all 4 mega-loads issued up-front on sync: queues stream back-to-back
        xt = []
        for i in range(NSW * NG):
            t = xpool.tile([P, NBINS * GW], F16, tag="x", name=f"x_{i}")
            nc.sync.dma_start(out=t, in_=x[i])
            xt.append(t)

        for s in range(NSW):
            for g in range(NG):
                T = xt[s * NG + g]
                last = s == NSW - 1 and g == NG - 1
                # The final tile is the critical path: run its c4..7 bins as
                # direct PE matmuls (start the moment the tile lands, in
                # parallel with DVE's c0..3 pair sums) instead of waiting on
                # more adds.
                if last:
                    for c in range(4, NBINS):
                        for j in range(CPG):
                            ch = g * CPG + j
                            nc.tensor.matmul(
                                psum_feat[s],
                                lhsT=T[:, c * GW + j * P:c * GW + (j + 1) * P],
                                rhs=w_sb[:, (s * NCHUNK + ch) * NF:(s * NCHUNK + ch + 1) * NF],
                                start=False,
                                stop=False,
                            )
                # c-pair sums on DVE; PE accumulates the partials in PSUM,
                # so no lvl1/root adds are needed.
                for m in range(2 if last else NPAIR):
                    u = upool.tile([P, GW], F16, tag="u", name=f"u_{s}_{g}_{m}")
                    nc.vector.tensor_add(
                        out=u,
                        in0=T[:, (2 * m) * GW:(2 * m + 1) * GW],
                        in1=T[:, (2 * m + 1) * GW:(2 * m + 2) * GW],
                    )
                    for j in range(CPG):
                        ch = g * CPG + j
                        nc.tensor.matmul(
                            psum_feat[s],
                            lhsT=u[:, j * P:(j + 1) * P],
                            rhs=w_sb[:, (s * NCHUNK + ch) * NF:(s * NCHUNK + ch + 1) * NF],
                            start=(g == 0 and m == 0 and j == 0),
                            stop=False,
                        )

            # bias via rank-1 matmul: ones[1, b].T @ bias[1, o]
            nc.tensor.matmul(
                psum_feat[s],
                lhsT=ones,
                rhs=bias_sb[:, s * NF:(s + 1) * NF],
                start=False,
                stop=True,
            )
            # LeakyReLU(v) = max(v, slope*v)  (slope < 1)
            tmp = upool.tile([P, NF], F32, tag="lrelu", name=f"lr_{s}")
            nc.vector.tensor_scalar_mul(tmp, psum_feat[s], SLOPE)
            nc.vector.tensor_max(
                out=out_sb[:, s * NF:(s + 1) * NF], in0=psum_feat[s], in1=tmp
            )
            # ship this subwindow's half right away from the drained sync ring
            nc.sync.dma_start(
                out=out[:, s * NF:(s + 1) * NF], in_=out_sb[:, s * NF:(s + 1) * NF]
            )


def _build():
    global _cached
    if _cached is not None:
        return _cached
    nc = bacc.Bacc(
        "TRN2",
        target_bir_lowering=False,
        debug=False,
        enable_asserts=False,
        num_devices=NCORES,
    )
    x_ap = nc.dram_tensor(
        "x", [NSW * NG, P, NBINS * GW], F16, kind="ExternalInput"
    ).ap()
    w_ap = nc.dram_tensor("w", [P, NSW * NCHUNK * NF], F16, kind="ExternalInput").ap()
    b_ap = nc.dram_tensor("bias", [1, OUT_F], F16, kind="ExternalInput").ap()
    out_ap = nc.dram_tensor("out", [BS, OUT_F], F32, kind="ExternalOutput").ap()
    with tile.TileContext(nc, trace_sim=False) as tc:
        _kernel_body(tc, x_ap, w_ap, b_ap, out_ap)
    nc.compile()
    _cached = nc
    return nc


def kernel(x, W, b):
    global last_results
    assert x.shape == (B, 1, NSW * NBINS * NDCT, HW, HW), x.shape
    nc = _build()

    # Host-side folding of the DCT matrices into the conv weights (tiny).
    Ct = _dct2(NDCT)                       # [f, t]
    Cs = _dct2(HW)                         # [p, i]
    Weff = np.einsum(
        "ft,pi,qj,sofpq->sotij", Ct, Cs, Cs, W.astype(np.float64), optimize=True
    ) / float(NBINS)
    Weff_k = Weff.reshape(NSW, NF, K)      # [s, o, k]
    # device layout: w[p, s*NCHUNK*NF + ch*NF + o] = Weff_k[s, o, ch*128 + p]
    w_dev = np.ascontiguousarray(
        Weff_k.reshape(NSW, NF, NCHUNK, P).transpose(3, 0, 2, 1).reshape(P, NSW * NCHUNK * NF)
    ).astype(np.float16)
    bias_dev = np.ascontiguousarray(b.reshape(1, OUT_F)).astype(np.float16)

    x2 = x.reshape(B, NSW, NBINS, NG, CPG, P)  # (b, s, c, g, chin, kin)
    in_maps = []
    for i in range(NCORES):
        xs = x2[i * BS:(i + 1) * BS]
        # -> [s, g, kin, c, chin, b]: one contiguous [128, 8192] block per (s,g)
        xt = np.ascontiguousarray(
            xs.transpose(1, 3, 5, 2, 4, 0).astype(np.float16)
        ).reshape(NSW * NG, P, NBINS * GW)
        in_maps.append({"x": xt, "w": w_dev, "bias": bias_dev})
    res = run_bass_kernel_spmd(nc, in_maps, core_ids=list(range(NCORES)))
    last_results = res
    return np.concatenate([r["out"] for r in res.results], axis=0)
